# revision 65
# baseline (speedup 1.0000x reference)
"""Trainium2 Bass kernel for nn_GCNStacking: 3-layer dense-adjacency GraphConv.

Per batch element b (one per NeuronCore, B=8 = n_cores=8, pure data parallel):
    H = relu(A @ (X @ Wm0^T) + X @ Ws0^T + b0)
    H = relu(A @ (H @ Wm1^T) + H @ Ws1^T + b1)
    H =      A @ (H @ Wm2^T) + H @ Ws2^T + b2

Dataflow (per core), state kept transposed, Ht = H^T [C=64, N=2048]:
  - A streams from HBM as bf16 via SWDGE cast-DMA (gpsimd engine casts
    f32->bf16 inline), in half-slab pieces [128, 1024]; 4 chunks of
    pieces kept in flight so the HBM stream free-runs at ~400 GB/s.
  - A^T materialized in SBUF via PE transposes of the bf16 pieces
    (bf16 transpose sustains ~105ns/block vs ~215ns for f32's LOW_HIGH
    2-pass).  Each PSUM bank packs one piece's 8 j-blocks
    [128, 8jb, 128i] and gates on that single piece DMA, evacuated by
    one wide DVE/ACT copy per bank (2:1 split).
  - Aggregation Ot = (A@M)^T: lhsT = M-block [128,64] stationary,
    rhs = A^T-block [128,512] moving, col-packed pairs (even jb ->
    psum partitions 0:64, odd -> 64:128) accumulate into one PSUM bank;
    chunk g aggregates in the SAME pipeline iteration as its transposes.
  - Mprod batches 4 j-blocks per PSUM bank with a single wide ACT copy;
    layer l+1's Mprod for chunk g is emitted one chunk late so its wait
    on the evac chain never stalls the PE FIFO.
  - All PSUM tiles are full 2KB banks so no two pools share a bank
    (shared banks cause false serialization / PSUM collisions).
  - bias+relu evacuation on Scalar/Vector engines; final layer transposed
    back to natural [N, C] via PE and DMA'd out one chunk per DMA.

Known pitfalls encoded here (measured on HW):
  - x-bar DMA-transpose (transpose=True) serializes against ALL other
    DMA traffic (Tile + HW) — unusable while the A stream runs.
  - A single scattered-AP X load (2048 x 256B descriptors) straggles
    ~4-8us behind issue; 16 separate sync DMAs are even worse (~2us
    fixed cost each, serialized).  One scattered DMA issued first wins.
  - The tile scheduler orders each engine's stream by its own cost-model
    simulation; emission order is only a hint.  Keep producer->consumer
    distance short and gate on single DMAs where possible.

GCN_LOAD=f32 env falls back to HWDGE f32 loads + f32 PE transposes
(slower transposes but no SWDGE cast dependency).
"""
import sys

for _p in ("/opt/trn_rl_repo",):
    if _p not in sys.path:
        sys.path.insert(0, _p)

import numpy as np
import orjson

import concourse.bass as bass
import concourse.tile as tile
from concourse import mybir
from concourse.bass import _add_dep_helper as add_dep

f32 = mybir.dt.float32
bf16 = mybir.dt.bfloat16

import os as _os
LOAD_MODE = _os.environ.get("GCN_LOAD", "bf16")   # "bf16" (SWDGE cast) | "f32"

# ---------------------------------------------------------------------------
# Workaround: this walrus build accepts at most ONE embedded sync-wait per
# instruction ("Too many sync wait commands").  Split excess waits onto
# inserted NoOps (same engine, right before the host instruction).
# ---------------------------------------------------------------------------
_ws_ctr = [0]


def _split_waits_json(bir_bytes: bytes) -> bytes:
    d = orjson.loads(bir_bytes)
    changed = False
    for fn in d.get("functions", []):
        for blk in fn.get("blocks", []):
            out = []
            for inst in blk.get("instructions", []):
                si = inst.get("sync_info")
                waits = (si or {}).get("on_wait") or []
                eng = inst.get("engine")
                if len(waits) > 1 and eng and eng != "Unassigned":
                    changed = True
                    for w in waits[:-1]:
                        _ws_ctr[0] += 1
                        out.append({
                            "name": f"I-wsplit-{_ws_ctr[0]}",
                            "opcode": "NoOp",
                            "engine": eng,
                            "ins": [],
                            "outs": [],
                            "sync_info": {"on_wait": [w], "on_update": []},
                        })
                    si["on_wait"] = waits[-1:]
                out.append(inst)
            blk["instructions"] = out
    return orjson.dumps(d) if changed else bir_bytes


def _install_waitsplit():
    from concourse import bass2jax, bass_utils
    if getattr(bass_utils, "_waitsplit_installed", False):
        return
    orig = bass_utils.compile_bir_kernel

    def patched(bir_json, tmpdir, neff_name="file.neff"):
        return orig(_split_waits_json(bytes(bir_json)), tmpdir, neff_name=neff_name)

    bass_utils.compile_bir_kernel = patched
    bass2jax.compile_bir_kernel = patched
    bass_utils._waitsplit_installed = True


_install_waitsplit()

# ---------------------------------------------------------------------------
# Kernel builder
# ---------------------------------------------------------------------------
P = 128
C = 64
N_LAYERS = 3


def build_gcn(nn_nodes: int = 2048):
    """Build the single-core Bass program; the same program runs SPMD on all
    8 cores with per-core (per-batch) inputs."""
    NN = nn_nodes
    NB = NN // P            # node blocks (16)
    CH = 512                # aggregation i-chunk (one PSUM bank of f32)
    IC = NN // CH           # i-chunks (4)
    HP = NN // 2            # half-slab piece width (j) for the A loads
    NHALF = 2
    SLABS_PER_CHUNK = CH // P   # 4

    use_bf16_load = LOAD_MODE == "bf16"
    adt = bf16 if use_bf16_load else f32   # dtype of staged A pieces
    JBG = 2 if use_bf16_load else 1        # j-blocks per transpose PSUM bank
    SLAB_BUFS = 32 if use_bf16_load else 16

    nc = bass.Bass()
    X_in = nc.declare_dram_parameter("X", [NN, C], f32, isOutput=False)
    A_in = nc.declare_dram_parameter("A", [NN, NN], f32, isOutput=False)
    W_in = {}
    b_in = {}
    for l in range(N_LAYERS):
        W_in[(l, "m")] = nc.declare_dram_parameter(f"Wm{l}", [C, C], f32, isOutput=False)
        W_in[(l, "s")] = nc.declare_dram_parameter(f"Ws{l}", [C, C], f32, isOutput=False)
        b_in[l] = nc.declare_dram_parameter(f"b{l}", [C], f32, isOutput=False)
    H_out = nc.declare_dram_parameter("H", [NN, C], f32, isOutput=True)

    with tile.TileContext(nc) as tc:
        with (
            tc.tile_pool(name="const", bufs=1) as const,
            tc.tile_pool(name="ht_pool", bufs=2) as ht_pool,
            tc.tile_pool(name="mn_pool", bufs=2) as mn_pool,
            tc.tile_pool(name="slab_pool", bufs=28) as slab_pool,
            tc.tile_pool(name="slab32_pool", bufs=10) as slab32_pool,
            tc.tile_pool(name="u_pool", bufs=3) as u_pool,
            tc.tile_pool(name="hb_pool", bufs=2) as hb_pool,
            tc.tile_pool(name="ps_tra", bufs=2, space="PSUM") as ps_tra,
            tc.tile_pool(name="ps_trx", bufs=1, space="PSUM") as ps_trx,
            tc.tile_pool(name="ps_o", bufs=2, space="PSUM") as ps_o,
            tc.tile_pool(name="ps_m", bufs=3, space="PSUM") as ps_m,
        ):
            # ---- A prefetch (first pieces before anything else on the Q7
            # so the HBM stream starts as early as possible) ---------------
            pieces = {}

            def a_load(g, h, si):
                # Loads split across BOTH DMA queues: during the load phase
                # the SDMA engines are ~50% idle — the stream is gated by
                # the 8-semaphore-per-queue recycle coupled to PE
                # consumption, so two queues nearly double the issue rate.
                # h=0 pieces: SWDGE (gpsimd) with inline f32->bf16 cast.
                # h=1 pieces: HWDGE (sync) f32 + a DVE/ACT cast pass.
                a_pc = slab_pool.tile([P, HP], adt, name="a_pc", tag="aslab")
                s = g * SLABS_PER_CHUNK + si
                src = A_in[s * P:(s + 1) * P, h * HP:(h + 1) * HP]
                if not use_bf16_load:
                    d = nc.sync.dma_start(a_pc, src)
                elif h == 0:
                    d = nc.gpsimd.dma_start(a_pc, src)
                else:
                    a_f = slab32_pool.tile([P, HP], f32, name="a_f",
                                           tag="aslab32")
                    nc.sync.dma_start(a_f, src)
                    if si % 2 == 0:
                        d = nc.vector.tensor_copy(a_pc, a_f)
                    else:
                        d = nc.scalar.copy(a_pc, a_f)
                pieces[(g, h, si)] = (a_pc, d)

            for si in range(2):
                a_load(0, 0, si)

            # ---- phase 0: constants, X^T, W^T -----------------------------
            ident = const.tile([P, P], f32, name="ident")
            id_i1 = nc.gpsimd.memset(ident, 0.0)
            id_i2 = nc.gpsimd.affine_select(
                out=ident, in_=ident,
                compare_op=mybir.AluOpType.not_equal,
                fill=1.0, base=0, pattern=[[-1, P]], channel_multiplier=1,
            )
            # bf16 identity for the bf16 A-transposes, built directly on
            # gpsimd (no DVE hop before the warmup can start)
            ident_bf = const.tile([P, P], bf16, name="ident_bf")
            idb0 = nc.gpsimd.memset(ident_bf, 0.0)
            idb = nc.gpsimd.affine_select(
                out=ident_bf, in_=ident_bf,
                compare_op=mybir.AluOpType.not_equal,
                fill=1.0, base=0, pattern=[[-1, P]], channel_multiplier=1,
            )

            # X loads first on the sync queue (one scattered-AP DMA; its
            # receipt straggles ~4-8us behind issue under the A-load
            # fabric contention, so it must go out as early as possible)
            x_sb = const.tile([P, NB, C], f32, name="x_sb")
            x_dma = nc.sync.dma_start(
                x_sb, X_in[:].rearrange("(nb p) c -> p nb c", p=P))

            w_stage = {}
            w_dmas = []
            for l in range(N_LAYERS):
                for kind in ("m", "s"):
                    wst = const.tile([C, C], f32, name=f"wst_{l}{kind}")
                    w_dmas.append(nc.sync.dma_start(wst, W_in[(l, kind)][:]))
                    w_stage[(l, kind)] = wst
            b_sb = {}
            for l in range(N_LAYERS):
                bt = const.tile([C, 1], f32, name=f"b_sb{l}")
                nc.sync.dma_start(bt, b_in[l][:].rearrange("(p o) -> p o", o=1))
                b_sb[l] = bt

            # ---- rest of the A prefetch (chunks 0-2) ----------------------
            for g in range(3):
                for h in range(NHALF):
                    for si in range(SLABS_PER_CHUNK):
                        if (g, h, si) not in pieces:
                            a_load(g, h, si)

            # gates: PE nops absorbing phase-0 input waits so the f32
            # transposes below carry at most one embedded wait each.
            # X and W gate separately so X^T (the critical path into
            # mn1) doesn't wait for the weight DMAs.
            gate0x = nc.tensor.nop(nofuse=True)
            for d in (id_i1, id_i2, x_dma):
                add_dep(gate0x.ins, d.ins, True, "phase0 x gate")
            gate0w = nc.tensor.nop(nofuse=True)
            for d in w_dmas:
                add_dep(gate0w.ins, d.ins, True, "phase0 w gate")

            # warm-up matmuls: engage the PE HAM clock-gate (~3.4us of
            # sustained activity -> 2.4 GHz) before real data arrives
            warm_gate = nc.tensor.nop(nofuse=True)
            add_dep(warm_gate.ins, idb.ins, True, "warmup gate")
            pwarm = ps_m.tile([P, CH], f32, name="pwarm", tag="m")
            for wi in range(24):
                wmm = nc.tensor.matmul(pwarm[:P, :C], ident_bf, ident_bf[:, :C],
                                       start=True, stop=True,
                                       skip_group_check=True)
                if wi == 0:
                    add_dep(wmm.ins, warm_gate.ins, False, "after warmup gate")

            # Ht[l]: transposed state [C, NN] bf16; Ht[0] = X^T.
            # X^T packs 4 transposed blocks per PSUM bank -> 4 wide
            # CAST copies instead of 16 narrow ones.
            Ht = [ht_pool.tile([C, NN], bf16, name=f"Ht{l}", tag="ht")
                  for l in range(N_LAYERS)]
            for nq in range(NB // 4):
                pt = ps_trx.tile([P, CH], f32, name="pt_x", tag="trx")
                for k in range(4):
                    t = nc.tensor.transpose(pt[:C, k * P:(k + 1) * P],
                                            x_sb[:, nq * 4 + k, :], ident)
                    if k == 0:
                        add_dep(t.ins, gate0x.ins, False, "after gate0x")
                nc.vector.tensor_copy(Ht[0][:, nq * CH:(nq + 1) * CH],
                                      pt[:C, :])

            wT = {}

            def emit_wT(l, kind):
                pw = ps_trx.tile([P, CH], f32, name="pt_w", tag="trx")
                t = nc.tensor.transpose(pw[:C, :C], w_stage[(l, kind)],
                                        ident[:C, :C])
                add_dep(t.ins, gate0w.ins, False, "after gate0w")
                wt = const.tile([C, C], bf16, name=f"wT_{l}{kind}")
                nc.vector.tensor_copy(wt, pw[:C, :C])
                wT[(l, kind)] = wt

            # Wm0^T first — it gates the mn1 chain, which gates L0 agg;
            # the other five W^T follow after mn1 is queued.
            emit_wT(0, "m")

            # resident A^T [j-partition, j-block, i] bf16
            ATr = const.tile([P, NB, NN], bf16, name="ATr")

            def emit_mprod(l, mn, jbs=None):
                """M_l natural [N, C] blocks: lhsT = Ht[l] block, rhs = WmT.
                Four j-blocks share one PSUM bank and evacuate with a single
                wide ACT copy (fixed cost dominates the tiny copies)."""
                jbs = list(range(NB) if jbs is None else jbs)
                for q0 in range(0, len(jbs), 4):
                    grp = jbs[q0:q0 + 4]
                    pm = ps_m.tile([P, len(grp), C], f32, name="pm", tag="m",
                                   padded_shape=(P, 4, 2 * C))
                    for k, jb in enumerate(grp):
                        nc.tensor.matmul(pm[:, k, :],
                                         Ht[l][:, jb * P:(jb + 1) * P],
                                         wT[(l, "m")], start=True, stop=True,
                                         skip_group_check=True)
                    # Mn copies on Scalar (ACT) to keep DVE free
                    nc.scalar.copy(mn[:, grp[0]:grp[0] + len(grp), :], pm)

            def emit_evac(l, g, po):
                # col-packed halves: out = po[0:64] + po[64:128] + b.
                # Engines read at most one non-scalar PSUM input per op.
                v = u_pool.tile([C, CH], f32, name="v", tag="v")
                nc.scalar.activation(v, po[C:2 * C, :],
                                     mybir.ActivationFunctionType.Identity,
                                     bias=b_sb[l], scale=1.0)
                if l < N_LAYERS - 1:
                    u = u_pool.tile([C, CH], f32, name="u", tag="u")
                    nc.vector.tensor_tensor(u, po[:C, :], v,
                                            mybir.AluOpType.add)
                    nc.vector.tensor_scalar(
                        Ht[l + 1][:, g * CH:(g + 1) * CH], u,
                        0.0, None, mybir.AluOpType.max)
                    return
                ho = u_pool.tile([C, CH], f32, name="ho", tag="ho")
                nc.vector.tensor_tensor(ho, po[:C, :], v,
                                        mybir.AluOpType.add)
                # final layer: back to natural layout and out to DRAM,
                # 4 transposed blocks packed into one PSUM bank, one copy
                # and one DMA per chunk
                ph = ps_trx.tile([P, CH // P, P], f32, name="ph", tag="trx")
                for k in range(CH // P):
                    nc.tensor.transpose(ph[:, k, :C],
                                        ho[:, k * P:(k + 1) * P],
                                        ident[:C, :C])
                hb = hb_pool.tile([P, CH // P, C], f32, name="hb", tag="hb")
                nc.vector.tensor_copy(hb, ph[:, :, :C])
                r0 = g * CH
                nc.sync.dma_start(
                    H_out[r0:r0 + CH, :].rearrange("(k p) c -> p k c", p=P),
                    hb)

            # ---- layer 1 Mprod, pipelined with the A load/transpose -------
            # Only the first half of mn1 is emitted up front: agg(0,0,h0)
            # needs mn1[0:8]; the second half and the remaining W^T slot in
            # behind the first transpose group.
            mn1 = mn_pool.tile([P, NB, C], bf16, name="mn", tag="mn")
            with tc.high_priority():
                emit_mprod(0, mn1, range(NB // 2))

            def cast_copy(eng_idx, dst, srcp):
                # 2:1 DVE:ACT — ACT copies are ~1.6x slower but ACT has
                # slack during the L0 phase
                if eng_idx % 3 != 2:
                    nc.vector.tensor_copy(dst, srcp)
                else:
                    nc.scalar.copy(dst, srcp)

            _cc = [0]

            def emit_transpose_half(g, h):
                """Transpose the 8 j-blocks of half h, one PIECE (slab) per
                PSUM bank: each group gates on a single piece DMA, so
                transposes start as soon as each piece lands and the slab
                buffer frees right after its own 8 transposes."""
                idm = ident_bf if use_bf16_load else ident
                nhb = NB // 2    # j-blocks per half (8)
                gjb = nhb if use_bf16_load else nhb // 2   # jb per PSUM bank
                for si in range(SLABS_PER_CHUNK):
                    src, dma = pieces[(g, h, si)]
                    s = g * SLABS_PER_CHUNK + si
                    for sub in range(nhb // gjb):
                        pt = ps_tra.tile([P, gjb, P], adt, name="pt_a",
                                         tag="tr")
                        for jj in range(gjb):
                            ljb = sub * gjb + jj
                            t = nc.tensor.transpose(
                                pt[:, jj, :],
                                src[:, ljb * P:(ljb + 1) * P], idm)
                            if jj == 0 and sub == 0:
                                add_dep(t.ins, dma.ins, True, "piece wait")
                        _cc[0] += 1
                        jb0 = h * nhb + sub * gjb
                        cast_copy(_cc[0],
                                  ATr[:, jb0:jb0 + gjb, s * P:(s + 1) * P],
                                  pt)

            # agg chunk split into two emission halves for interleaving
            open_po = {}

            def emit_agg_half(l, g, mn, half):
                cs = slice(g * CH, (g + 1) * CH)
                # col-packed: even j-blocks -> partitions 0:64, odd ->
                # 64:128; concurrent in the array's column groups
                if half == 0:
                    po = ps_o.tile([P, CH], f32, name="po", tag="o")
                    open_po[(l, g)] = po
                    jbs = range(0, NB // 2)
                else:
                    po = open_po.pop((l, g))
                    jbs = range(NB // 2, NB)
                for jb in jbs:
                    hh = jb % 2
                    nc.tensor.matmul(
                        po[hh * C:(hh + 1) * C, :], mn[:, jb, :],
                        ATr[:, jb, cs],
                        start=(jb < 2), stop=(hh == 1 and jb == NB - 1),
                        skip_group_check=True)
                if half == 1:
                    nc.tensor.matmul(
                        po[:C, :], wT[(l, "s")], Ht[l][:, cs],
                        start=False, stop=True, skip_group_check=True)
                    emit_evac(l, g, po)

            # ---- layer 1, pipelined with the A load/transpose -------------
            # Mprod for layer l+1 chunk g is emitted one chunk LATE (after
            # agg chunk g+1) so its wait on the evac chain of chunk g is
            # already satisfied when the PE reaches it — no FIFO stall.
            mns = {0: mn1}
            for l in range(1, N_LAYERS):
                mns[l] = mn_pool.tile([P, NB, C], bf16, name="mn", tag="mn")
            JPC = NB // IC      # j-blocks per chunk (4)

            # agg chunk g consumes exactly the ATr columns chunk g's
            # transposes produce, so it runs in the SAME iteration,
            # half-by-half — no cross-chunk lag on the PE FIFO.
            for g in range(IC):
                if g + 3 < IC:      # keep 4 chunks of pieces in flight
                    for h in range(NHALF):
                        for si in range(SLABS_PER_CHUNK):
                            a_load(g + 3, h, si)
                for h in range(NHALF):
                    emit_transpose_half(g, h)
                    if g == 0 and h == 0:
                        # finish mn1 + the five non-critical W^T in the
                        # shadow of the first transpose group
                        with tc.high_priority():
                            emit_mprod(0, mn1, range(NB // 2, NB))
                        for l in range(N_LAYERS):
                            for kind in ("m", "s"):
                                if (l, kind) not in wT:
                                    emit_wT(l, kind)
                    emit_agg_half(0, g, mn1, h)
                if g >= 1:
                    emit_mprod(1, mns[1], range((g - 1) * JPC, g * JPC))
            emit_mprod(1, mns[1], range((IC - 1) * JPC, IC * JPC))

            # ---- layers 2..3 ---------------------------------------------
            for l in range(1, N_LAYERS):
                mn = mns[l]
                for g in range(IC):
                    emit_agg_half(l, g, mn, 0)
                    emit_agg_half(l, g, mn, 1)
                    if l + 1 < N_LAYERS and g >= 1:
                        emit_mprod(l + 1, mns[l + 1],
                                   range((g - 1) * JPC, g * JPC))
                if l + 1 < N_LAYERS:
                    emit_mprod(l + 1, mns[l + 1],
                               range((IC - 1) * JPC, IC * JPC))

    return nc


# ---------------------------------------------------------------------------
# Harness entry point
# ---------------------------------------------------------------------------
_NC_CACHE = {}


def _get_nc(nn_nodes):
    if nn_nodes not in _NC_CACHE:
        _NC_CACHE[nn_nodes] = build_gcn(nn_nodes)
    return _NC_CACHE[nn_nodes]


def kernel(X, A, Wm0, Ws0, b0, Wm1, Ws1, b1, Wm2, Ws2, b2, _trace=False):
    from concourse.bass_utils import run_bass_kernel_spmd

    X = np.ascontiguousarray(np.asarray(X, dtype=np.float32))
    A = np.ascontiguousarray(np.asarray(A, dtype=np.float32))
    B, NN, _C = X.shape
    assert B == 8, f"expected batch 8 (one per core), got {B}"

    shared = {
        "Wm0": np.ascontiguousarray(np.asarray(Wm0, np.float32)),
        "Ws0": np.ascontiguousarray(np.asarray(Ws0, np.float32)),
        "b0": np.ascontiguousarray(np.asarray(b0, np.float32)),
        "Wm1": np.ascontiguousarray(np.asarray(Wm1, np.float32)),
        "Ws1": np.ascontiguousarray(np.asarray(Ws1, np.float32)),
        "b1": np.ascontiguousarray(np.asarray(b1, np.float32)),
        "Wm2": np.ascontiguousarray(np.asarray(Wm2, np.float32)),
        "Ws2": np.ascontiguousarray(np.asarray(Ws2, np.float32)),
        "b2": np.ascontiguousarray(np.asarray(b2, np.float32)),
    }
    nc = _get_nc(NN)
    in_maps = [dict(shared, X=X[b], A=A[b]) for b in range(B)]
    res = run_bass_kernel_spmd(nc, in_maps, core_ids=list(range(B)),
                               trace=_trace)
    out = np.stack([res.results[b]["H"] for b in range(B)], axis=0)
    if _trace:
        return out, res
    return out


# revision 66
# speedup vs baseline: 1.1378x; 1.1378x over previous
"""Trainium2 Bass kernel for nn_GCNStacking: 3-layer dense-adjacency GraphConv.

Per batch element b (one per NeuronCore, B=8 = n_cores=8, pure data parallel):
    H = relu(A @ (X @ Wm0^T) + X @ Ws0^T + b0)
    H = relu(A @ (H @ Wm1^T) + H @ Ws1^T + b1)
    H =      A @ (H @ Wm2^T) + H @ Ws2^T + b2

Dataflow (per core), state kept transposed, Ht = H^T [C=64, N=2048]:
  - A streams from HBM as bf16 via SWDGE cast-DMA (gpsimd engine casts
    f32->bf16 inline), in half-slab pieces [128, 1024]; 4 chunks of
    pieces kept in flight so the HBM stream free-runs at ~400 GB/s.
  - A^T materialized in SBUF via PE transposes of the bf16 pieces
    (bf16 transpose sustains ~105ns/block vs ~215ns for f32's LOW_HIGH
    2-pass).  Each PSUM bank packs one piece's 8 j-blocks
    [128, 8jb, 128i] and gates on that single piece DMA, evacuated by
    one wide DVE/ACT copy per bank (2:1 split).
  - Aggregation Ot = (A@M)^T: lhsT = M-block [128,64] stationary,
    rhs = A^T-block [128,512] moving, col-packed pairs (even jb ->
    psum partitions 0:64, odd -> 64:128) accumulate into one PSUM bank;
    chunk g aggregates in the SAME pipeline iteration as its transposes.
  - Mprod batches 4 j-blocks per PSUM bank with a single wide ACT copy;
    layer l+1's Mprod for chunk g is emitted one chunk late so its wait
    on the evac chain never stalls the PE FIFO.
  - All PSUM tiles are full 2KB banks so no two pools share a bank
    (shared banks cause false serialization / PSUM collisions).
  - bias+relu evacuation on Scalar/Vector engines; final layer transposed
    back to natural [N, C] via PE and DMA'd out one chunk per DMA.

Known pitfalls encoded here (measured on HW):
  - x-bar DMA-transpose (transpose=True) serializes against ALL other
    DMA traffic (Tile + HW) — unusable while the A stream runs.
  - A single scattered-AP X load (2048 x 256B descriptors) straggles
    ~4-8us behind issue; 16 separate sync DMAs are even worse (~2us
    fixed cost each, serialized).  One scattered DMA issued first wins.
  - The tile scheduler orders each engine's stream by its own cost-model
    simulation; emission order is only a hint.  Keep producer->consumer
    distance short and gate on single DMAs where possible.

GCN_LOAD=f32 env falls back to HWDGE f32 loads + f32 PE transposes
(slower transposes but no SWDGE cast dependency).
"""
import sys

for _p in ("/opt/trn_rl_repo",):
    if _p not in sys.path:
        sys.path.insert(0, _p)

import numpy as np
import orjson

import concourse.bass as bass
import concourse.tile as tile
from concourse import mybir
from concourse.bass import _add_dep_helper as add_dep

f32 = mybir.dt.float32
bf16 = mybir.dt.bfloat16

import os as _os
LOAD_MODE = _os.environ.get("GCN_LOAD", "bf16")   # "bf16" (SWDGE cast) | "f32"

# ---------------------------------------------------------------------------
# Workaround: this walrus build accepts at most ONE embedded sync-wait per
# instruction ("Too many sync wait commands").  Split excess waits onto
# inserted NoOps (same engine, right before the host instruction).
# ---------------------------------------------------------------------------
_ws_ctr = [0]


def _split_waits_json(bir_bytes: bytes) -> bytes:
    d = orjson.loads(bir_bytes)
    changed = False
    for fn in d.get("functions", []):
        for blk in fn.get("blocks", []):
            out = []
            for inst in blk.get("instructions", []):
                si = inst.get("sync_info")
                waits = (si or {}).get("on_wait") or []
                eng = inst.get("engine")
                if len(waits) > 1 and eng and eng != "Unassigned":
                    changed = True
                    for w in waits[:-1]:
                        _ws_ctr[0] += 1
                        out.append({
                            "name": f"I-wsplit-{_ws_ctr[0]}",
                            "opcode": "NoOp",
                            "engine": eng,
                            "ins": [],
                            "outs": [],
                            "sync_info": {"on_wait": [w], "on_update": []},
                        })
                    si["on_wait"] = waits[-1:]
                out.append(inst)
            blk["instructions"] = out
    return orjson.dumps(d) if changed else bir_bytes


def _install_waitsplit():
    from concourse import bass2jax, bass_utils
    if getattr(bass_utils, "_waitsplit_installed", False):
        return
    orig = bass_utils.compile_bir_kernel

    def patched(bir_json, tmpdir, neff_name="file.neff"):
        return orig(_split_waits_json(bytes(bir_json)), tmpdir, neff_name=neff_name)

    bass_utils.compile_bir_kernel = patched
    bass2jax.compile_bir_kernel = patched
    bass_utils._waitsplit_installed = True


_install_waitsplit()

# ---------------------------------------------------------------------------
# Kernel builder
# ---------------------------------------------------------------------------
P = 128
C = 64
N_LAYERS = 3


def build_gcn(nn_nodes: int = 2048):
    """Build the single-core Bass program; the same program runs SPMD on all
    8 cores with per-core (per-batch) inputs."""
    NN = nn_nodes
    NB = NN // P            # node blocks (16)
    CH = 512                # aggregation i-chunk (one PSUM bank of f32)
    IC = NN // CH           # i-chunks (4)
    HP = NN // 2            # half-slab piece width (j) for the A loads
    NHALF = 2
    SLABS_PER_CHUNK = CH // P   # 4

    use_bf16_load = LOAD_MODE == "bf16"
    adt = bf16 if use_bf16_load else f32   # dtype of staged A pieces
    JBG = 2 if use_bf16_load else 1        # j-blocks per transpose PSUM bank
    SLAB_BUFS = 32 if use_bf16_load else 16

    nc = bass.Bass()
    X_in = nc.declare_dram_parameter("X", [NN, C], f32, isOutput=False)
    A_in = nc.declare_dram_parameter("A", [NN, NN], f32, isOutput=False)
    W_in = {}
    b_in = {}
    for l in range(N_LAYERS):
        W_in[(l, "m")] = nc.declare_dram_parameter(f"Wm{l}", [C, C], f32, isOutput=False)
        W_in[(l, "s")] = nc.declare_dram_parameter(f"Ws{l}", [C, C], f32, isOutput=False)
        b_in[l] = nc.declare_dram_parameter(f"b{l}", [C], f32, isOutput=False)
    H_out = nc.declare_dram_parameter("H", [NN, C], f32, isOutput=True)

    with tile.TileContext(nc) as tc:
        with (
            tc.tile_pool(name="const", bufs=1) as const,
            tc.tile_pool(name="ht_pool", bufs=2) as ht_pool,
            tc.tile_pool(name="mn_pool", bufs=2) as mn_pool,
            tc.tile_pool(name="slab_pool", bufs=SLAB_BUFS) as slab_pool,
            tc.tile_pool(name="u_pool", bufs=3) as u_pool,
            tc.tile_pool(name="hb_pool", bufs=2) as hb_pool,
            tc.tile_pool(name="ps_tra", bufs=2, space="PSUM") as ps_tra,
            tc.tile_pool(name="ps_trx", bufs=1, space="PSUM") as ps_trx,
            tc.tile_pool(name="ps_o", bufs=2, space="PSUM") as ps_o,
            tc.tile_pool(name="ps_m", bufs=3, space="PSUM") as ps_m,
        ):
            # ---- A prefetch (first pieces before anything else on the Q7
            # so the HBM stream starts as early as possible) ---------------
            pieces = {}

            def a_load(g, h, si):
                a_pc = slab_pool.tile([P, HP], adt, name="a_pc", tag="aslab")
                s = g * SLABS_PER_CHUNK + si
                src = A_in[s * P:(s + 1) * P, h * HP:(h + 1) * HP]
                if use_bf16_load:
                    d = nc.gpsimd.dma_start(a_pc, src)
                else:
                    d = nc.sync.dma_start(a_pc, src)
                pieces[(g, h, si)] = (a_pc, d)

            for si in range(2):
                a_load(0, 0, si)

            # ---- phase 0: constants, X^T, W^T -----------------------------
            ident = const.tile([P, P], f32, name="ident")
            id_i1 = nc.gpsimd.memset(ident, 0.0)
            id_i2 = nc.gpsimd.affine_select(
                out=ident, in_=ident,
                compare_op=mybir.AluOpType.not_equal,
                fill=1.0, base=0, pattern=[[-1, P]], channel_multiplier=1,
            )
            # bf16 identity for the bf16 A-transposes, built directly on
            # gpsimd (no DVE hop before the warmup can start)
            ident_bf = const.tile([P, P], bf16, name="ident_bf")
            idb0 = nc.gpsimd.memset(ident_bf, 0.0)
            idb = nc.gpsimd.affine_select(
                out=ident_bf, in_=ident_bf,
                compare_op=mybir.AluOpType.not_equal,
                fill=1.0, base=0, pattern=[[-1, P]], channel_multiplier=1,
            )

            # X loads first on the sync queue (one scattered-AP DMA; its
            # receipt straggles ~4-8us behind issue under the A-load
            # fabric contention, so it must go out as early as possible)
            x_sb = const.tile([P, NB, C], f32, name="x_sb")
            x_dma = nc.sync.dma_start(
                x_sb, X_in[:].rearrange("(nb p) c -> p nb c", p=P))

            w_stage = {}
            w_dmas = []
            for l in range(N_LAYERS):
                for kind in ("m", "s"):
                    wst = const.tile([C, C], f32, name=f"wst_{l}{kind}")
                    w_dmas.append(nc.sync.dma_start(wst, W_in[(l, kind)][:]))
                    w_stage[(l, kind)] = wst
            b_sb = {}
            for l in range(N_LAYERS):
                bt = const.tile([C, 1], f32, name=f"b_sb{l}")
                nc.sync.dma_start(bt, b_in[l][:].rearrange("(p o) -> p o", o=1))
                b_sb[l] = bt

            # ---- rest of the A prefetch (chunks 0-2) ----------------------
            for g in range(3):
                for h in range(NHALF):
                    for si in range(SLABS_PER_CHUNK):
                        if (g, h, si) not in pieces:
                            a_load(g, h, si)

            # gates: PE nops absorbing phase-0 input waits so the f32
            # transposes below carry at most one embedded wait each.
            # X and W gate separately so X^T (the critical path into
            # mn1) doesn't wait for the weight DMAs.
            gate0x = nc.tensor.nop(nofuse=True)
            for d in (id_i1, id_i2, x_dma):
                add_dep(gate0x.ins, d.ins, True, "phase0 x gate")
            gate0w = nc.tensor.nop(nofuse=True)
            for d in w_dmas:
                add_dep(gate0w.ins, d.ins, True, "phase0 w gate")

            # warm-up matmuls: engage the PE HAM clock-gate (~3.4us of
            # sustained activity -> 2.4 GHz) before real data arrives
            warm_gate = nc.tensor.nop(nofuse=True)
            add_dep(warm_gate.ins, idb.ins, True, "warmup gate")
            pwarm = ps_m.tile([P, CH], f32, name="pwarm", tag="m")
            for wi in range(24):
                wmm = nc.tensor.matmul(pwarm[:P, :C], ident_bf, ident_bf[:, :C],
                                       start=True, stop=True,
                                       skip_group_check=True)
                if wi == 0:
                    add_dep(wmm.ins, warm_gate.ins, False, "after warmup gate")

            # Ht[l]: transposed state [C, NN] bf16; Ht[0] = X^T.
            # X^T packs 4 transposed blocks per PSUM bank -> 4 wide
            # CAST copies instead of 16 narrow ones.
            Ht = [ht_pool.tile([C, NN], bf16, name=f"Ht{l}", tag="ht")
                  for l in range(N_LAYERS)]
            for nq in range(NB // 4):
                pt = ps_trx.tile([P, CH], f32, name="pt_x", tag="trx")
                for k in range(4):
                    t = nc.tensor.transpose(pt[:C, k * P:(k + 1) * P],
                                            x_sb[:, nq * 4 + k, :], ident)
                    if k == 0:
                        add_dep(t.ins, gate0x.ins, False, "after gate0x")
                nc.vector.tensor_copy(Ht[0][:, nq * CH:(nq + 1) * CH],
                                      pt[:C, :])

            wT = {}

            def emit_wT(l, kind):
                pw = ps_trx.tile([P, CH], f32, name="pt_w", tag="trx")
                t = nc.tensor.transpose(pw[:C, :C], w_stage[(l, kind)],
                                        ident[:C, :C])
                add_dep(t.ins, gate0w.ins, False, "after gate0w")
                wt = const.tile([C, C], bf16, name=f"wT_{l}{kind}")
                nc.vector.tensor_copy(wt, pw[:C, :C])
                wT[(l, kind)] = wt

            # Wm0^T first — it gates the mn1 chain, which gates L0 agg;
            # the other five W^T follow after mn1 is queued.
            emit_wT(0, "m")

            # resident A^T [j-partition, j-block, i] bf16
            ATr = const.tile([P, NB, NN], bf16, name="ATr")

            def emit_mprod(l, mn, jbs=None):
                """M_l natural [N, C] blocks: lhsT = Ht[l] block, rhs = WmT.
                Four j-blocks share one PSUM bank and evacuate with a single
                wide ACT copy (fixed cost dominates the tiny copies)."""
                jbs = list(range(NB) if jbs is None else jbs)
                for q0 in range(0, len(jbs), 4):
                    grp = jbs[q0:q0 + 4]
                    pm = ps_m.tile([P, len(grp), C], f32, name="pm", tag="m",
                                   padded_shape=(P, 4, 2 * C))
                    for k, jb in enumerate(grp):
                        nc.tensor.matmul(pm[:, k, :],
                                         Ht[l][:, jb * P:(jb + 1) * P],
                                         wT[(l, "m")], start=True, stop=True,
                                         skip_group_check=True)
                    # Mn copies on Scalar (ACT) to keep DVE free
                    nc.scalar.copy(mn[:, grp[0]:grp[0] + len(grp), :], pm)

            def emit_evac(l, g, po):
                # col-packed halves: out = po[0:64] + po[64:128] + b.
                # Engines read at most one non-scalar PSUM input per op.
                v = u_pool.tile([C, CH], f32, name="v", tag="v")
                nc.scalar.activation(v, po[C:2 * C, :],
                                     mybir.ActivationFunctionType.Identity,
                                     bias=b_sb[l], scale=1.0)
                if l < N_LAYERS - 1:
                    u = u_pool.tile([C, CH], f32, name="u", tag="u")
                    nc.vector.tensor_tensor(u, po[:C, :], v,
                                            mybir.AluOpType.add)
                    nc.vector.tensor_scalar(
                        Ht[l + 1][:, g * CH:(g + 1) * CH], u,
                        0.0, None, mybir.AluOpType.max)
                    return
                ho = u_pool.tile([C, CH], f32, name="ho", tag="ho")
                nc.vector.tensor_tensor(ho, po[:C, :], v,
                                        mybir.AluOpType.add)
                # final layer: back to natural layout and out to DRAM,
                # 4 transposed blocks packed into one PSUM bank, one copy
                # and one DMA per chunk
                ph = ps_trx.tile([P, CH // P, P], f32, name="ph", tag="trx")
                for k in range(CH // P):
                    nc.tensor.transpose(ph[:, k, :C],
                                        ho[:, k * P:(k + 1) * P],
                                        ident[:C, :C])
                hb = hb_pool.tile([P, CH // P, C], f32, name="hb", tag="hb")
                nc.vector.tensor_copy(hb, ph[:, :, :C])
                r0 = g * CH
                nc.sync.dma_start(
                    H_out[r0:r0 + CH, :].rearrange("(k p) c -> p k c", p=P),
                    hb)

            # ---- layer 1 Mprod, pipelined with the A load/transpose -------
            # Only the first half of mn1 is emitted up front: agg(0,0,h0)
            # needs mn1[0:8]; the second half and the remaining W^T slot in
            # behind the first transpose group.
            mn1 = mn_pool.tile([P, NB, C], bf16, name="mn", tag="mn")
            with tc.high_priority():
                emit_mprod(0, mn1, range(NB // 2))

            def cast_copy(eng_idx, dst, srcp):
                # 2:1 DVE:ACT — ACT copies are ~1.6x slower but ACT has
                # slack during the L0 phase
                if eng_idx % 3 != 2:
                    nc.vector.tensor_copy(dst, srcp)
                else:
                    nc.scalar.copy(dst, srcp)

            _cc = [0]

            def emit_transpose_half(g, h):
                """Transpose the 8 j-blocks of half h, one PIECE (slab) per
                PSUM bank: each group gates on a single piece DMA, so
                transposes start as soon as each piece lands and the slab
                buffer frees right after its own 8 transposes."""
                idm = ident_bf if use_bf16_load else ident
                nhb = NB // 2    # j-blocks per half (8)
                gjb = nhb if use_bf16_load else nhb // 2   # jb per PSUM bank
                for si in range(SLABS_PER_CHUNK):
                    src, dma = pieces[(g, h, si)]
                    s = g * SLABS_PER_CHUNK + si
                    for sub in range(nhb // gjb):
                        pt = ps_tra.tile([P, gjb, P], adt, name="pt_a",
                                         tag="tr")
                        for jj in range(gjb):
                            ljb = sub * gjb + jj
                            t = nc.tensor.transpose(
                                pt[:, jj, :],
                                src[:, ljb * P:(ljb + 1) * P], idm)
                            if jj == 0 and sub == 0:
                                add_dep(t.ins, dma.ins, True, "piece wait")
                        _cc[0] += 1
                        jb0 = h * nhb + sub * gjb
                        cast_copy(_cc[0],
                                  ATr[:, jb0:jb0 + gjb, s * P:(s + 1) * P],
                                  pt)

            # agg chunk split into two emission halves for interleaving
            open_po = {}

            def emit_agg_half(l, g, mn, half):
                cs = slice(g * CH, (g + 1) * CH)
                # col-packed: even j-blocks -> partitions 0:64, odd ->
                # 64:128; concurrent in the array's column groups
                if half == 0:
                    po = ps_o.tile([P, CH], f32, name="po", tag="o")
                    open_po[(l, g)] = po
                    jbs = range(0, NB // 2)
                else:
                    po = open_po.pop((l, g))
                    jbs = range(NB // 2, NB)
                for jb in jbs:
                    hh = jb % 2
                    nc.tensor.matmul(
                        po[hh * C:(hh + 1) * C, :], mn[:, jb, :],
                        ATr[:, jb, cs],
                        start=(jb < 2), stop=(hh == 1 and jb == NB - 1),
                        skip_group_check=True)
                if half == 1:
                    nc.tensor.matmul(
                        po[:C, :], wT[(l, "s")], Ht[l][:, cs],
                        start=False, stop=True, skip_group_check=True)
                    emit_evac(l, g, po)

            # ---- layer 1, pipelined with the A load/transpose -------------
            # Mprod for layer l+1 chunk g is emitted one chunk LATE (after
            # agg chunk g+1) so its wait on the evac chain of chunk g is
            # already satisfied when the PE reaches it — no FIFO stall.
            mns = {0: mn1}
            for l in range(1, N_LAYERS):
                mns[l] = mn_pool.tile([P, NB, C], bf16, name="mn", tag="mn")
            JPC = NB // IC      # j-blocks per chunk (4)

            # agg chunk g consumes exactly the ATr columns chunk g's
            # transposes produce, so it runs in the SAME iteration,
            # half-by-half — no cross-chunk lag on the PE FIFO.
            for g in range(IC):
                if g + 3 < IC:      # keep 4 chunks of pieces in flight
                    for h in range(NHALF):
                        for si in range(SLABS_PER_CHUNK):
                            a_load(g + 3, h, si)
                for h in range(NHALF):
                    emit_transpose_half(g, h)
                    if g == 0 and h == 0:
                        # finish mn1 + the five non-critical W^T in the
                        # shadow of the first transpose group
                        with tc.high_priority():
                            emit_mprod(0, mn1, range(NB // 2, NB))
                        for l in range(N_LAYERS):
                            for kind in ("m", "s"):
                                if (l, kind) not in wT:
                                    emit_wT(l, kind)
                    emit_agg_half(0, g, mn1, h)
                if g >= 1:
                    emit_mprod(1, mns[1], range((g - 1) * JPC, g * JPC))
            emit_mprod(1, mns[1], range((IC - 1) * JPC, IC * JPC))

            # ---- layers 2..3 ---------------------------------------------
            for l in range(1, N_LAYERS):
                mn = mns[l]
                for g in range(IC):
                    emit_agg_half(l, g, mn, 0)
                    emit_agg_half(l, g, mn, 1)
                    if l + 1 < N_LAYERS and g >= 1:
                        emit_mprod(l + 1, mns[l + 1],
                                   range((g - 1) * JPC, g * JPC))
                if l + 1 < N_LAYERS:
                    emit_mprod(l + 1, mns[l + 1],
                               range((IC - 1) * JPC, IC * JPC))

    return nc


# ---------------------------------------------------------------------------
# Harness entry point
# ---------------------------------------------------------------------------
_NC_CACHE = {}


def _get_nc(nn_nodes):
    if nn_nodes not in _NC_CACHE:
        _NC_CACHE[nn_nodes] = build_gcn(nn_nodes)
    return _NC_CACHE[nn_nodes]


def kernel(X, A, Wm0, Ws0, b0, Wm1, Ws1, b1, Wm2, Ws2, b2, _trace=False):
    from concourse.bass_utils import run_bass_kernel_spmd

    X = np.ascontiguousarray(np.asarray(X, dtype=np.float32))
    A = np.ascontiguousarray(np.asarray(A, dtype=np.float32))
    B, NN, _C = X.shape
    assert B == 8, f"expected batch 8 (one per core), got {B}"

    shared = {
        "Wm0": np.ascontiguousarray(np.asarray(Wm0, np.float32)),
        "Ws0": np.ascontiguousarray(np.asarray(Ws0, np.float32)),
        "b0": np.ascontiguousarray(np.asarray(b0, np.float32)),
        "Wm1": np.ascontiguousarray(np.asarray(Wm1, np.float32)),
        "Ws1": np.ascontiguousarray(np.asarray(Ws1, np.float32)),
        "b1": np.ascontiguousarray(np.asarray(b1, np.float32)),
        "Wm2": np.ascontiguousarray(np.asarray(Wm2, np.float32)),
        "Ws2": np.ascontiguousarray(np.asarray(Ws2, np.float32)),
        "b2": np.ascontiguousarray(np.asarray(b2, np.float32)),
    }
    nc = _get_nc(NN)
    in_maps = [dict(shared, X=X[b], A=A[b]) for b in range(B)]
    res = run_bass_kernel_spmd(nc, in_maps, core_ids=list(range(B)),
                               trace=_trace)
    out = np.stack([res.results[b]["H"] for b in range(B)], axis=0)
    if _trace:
        return out, res
    return out
